# revision 3
# baseline (speedup 1.0000x reference)
"""Bass/Trainium2 kernel for nn_CrossAttentionFusion.

The reference is a pair of seq_len==1 multi-head cross-attentions. With a
single key position, softmax over the key axis is identically 1, so
attention reduces to the V projection:

    attended = (kv @ wv.T + bv) @ w_out.T + b_out
             = kv @ (w_out @ wv).T + (w_out @ bv + b_out)

i.e. one [B, D] x [D, D] GEMM per branch (plus a bias), with the two
effective weights computed on the host from the small projection matrices.

Device kernel (per core, data-parallel over batch), per 128-row batch tile:
  - DMA xa/xb tile in (fp32)
  - fp32 -> bf16 cast on the scalar (ACT) engine
  - transpose to K-major: one XBAR DMA transpose per branch (SBUF->SBUF),
    or PE transposes via identity matmul when KERNEL_PE_TRANSPOSE=1
  - 8-step PSUM-accumulated bf16 matmuls (N=512, fp32 accum) on PE
  - DVE bias-add PSUM->SBUF
  - DMA the [128, 2048] fp32 output tile out
"""

import os

import numpy as np

B, D = 65536, 1024
N_CORES = 8
BC = B // N_CORES  # 8192 rows per core
P = 128
KT = D // P  # 8 k-tiles

# Updated on every run when tracing is enabled via KERNEL_TRACE=1
LAST_EXEC_TIME_NS = None
LAST_RESULTS = None

_NC_CACHE = {}


def _build_nc(bc=BC, xbar=None):
    import concourse.bacc as bacc
    import concourse.mybir as mybir
    import concourse.tile as tile
    from concourse.masks import make_identity

    if xbar is None:
        xbar = os.environ.get("KERNEL_PE_TRANSPOSE", "0") != "1"

    f32 = mybir.dt.float32
    bf16 = mybir.dt.bfloat16
    n_tiles = bc // P

    nc = bacc.Bacc(
        "TRN2",
        target_bir_lowering=False,
        debug=False,
        enable_asserts=False,
        num_devices=N_CORES,
    )

    xa = nc.dram_tensor("xa", [bc, D], f32, kind="ExternalInput").ap()
    xb = nc.dram_tensor("xb", [bc, D], f32, kind="ExternalInput").ap()
    # wab/wba hold W_eff.T tiled K-major: w[p, ko, n] = W_eff.T[ko*128 + p, n]
    wab = nc.dram_tensor("wab", [P, KT, D], bf16, kind="ExternalInput").ap()
    wba = nc.dram_tensor("wba", [P, KT, D], bf16, kind="ExternalInput").ap()
    bias = nc.dram_tensor("bias", [1, 2 * D], f32, kind="ExternalInput").ap()
    out = nc.dram_tensor("out", [bc, 2 * D], f32, kind="ExternalOutput").ap()

    with tile.TileContext(nc) as tc:
        with (
            tc.tile_pool(name="const", bufs=1) as const_pool,
            tc.tile_pool(name="xin", bufs=4) as xin_pool,
            tc.tile_pool(name="xbf", bufs=3) as xbf_pool,
            tc.tile_pool(name="xt", bufs=3) as xt_pool,
            tc.tile_pool(name="osb", bufs=3) as out_pool,
            tc.tile_pool(name="opsum", bufs=2, space="PSUM") as opsum,
        ):
            if not xbar:
                tpsum = tc.alloc_tile_pool(name="tpsum", bufs=3, space="PSUM")
                identity = const_pool.tile([P, P], bf16)
                make_identity(nc, identity)
            bias_bc = const_pool.tile([P, 2 * D], f32)
            nc.sync.dma_start(bias_bc[:], bias.to_broadcast((P, 2 * D)))
            wab_sb = const_pool.tile([P, KT, D], bf16)
            nc.sync.dma_start(wab_sb[:], wab)
            wba_sb = const_pool.tile([P, KT, D], bf16)
            nc.sync.dma_start(wba_sb[:], wba)

            for i in range(n_tiles):
                xa_t = xin_pool.tile([P, D], f32, tag="xa", name="xa_t")
                nc.sync.dma_start(xa_t[:], xa[i * P : (i + 1) * P, :])
                xb_t = xin_pool.tile([P, D], f32, tag="xb", name="xb_t")
                nc.sync.dma_start(xb_t[:], xb[i * P : (i + 1) * P, :])
                out_sb = out_pool.tile([P, 2 * D], f32, tag="out", name="out_sb")

                # branch 0 (ab) consumes xb; branch 1 (ba) consumes xa
                for br, (x_t, w_sb) in enumerate(((xb_t, wab_sb), (xa_t, wba_sb))):
                    x_bf = xbf_pool.tile([P, D], bf16, tag=f"xbf{br}", name="x_bf")
                    nc.scalar.copy(x_bf[:], x_t[:])
                    xT = xt_pool.tile([P, KT, P], bf16, tag=f"xT{br}", name="xT")
                    if xbar:
                        # xT[p, ko, b] = x_bf[b, ko*128 + p]
                        nc.sync.dma_start_transpose(xT[:], x_bf[:])
                    else:
                        for kt in range(KT):
                            tp = tpsum.tile([P, P], bf16, tag="tp", name="tp")
                            nc.tensor.transpose(
                                tp[:], x_bf[:, kt * P : (kt + 1) * P], identity[:]
                            )
                            nc.vector.tensor_copy(xT[:, kt, :], tp[:])
                    for nh in range(2):
                        ps = opsum.tile([P, 512], f32, tag=f"ps{br}{nh}", name="ps")
                        for kt in range(KT):
                            nc.tensor.matmul(
                                ps[:],
                                lhsT=xT[:, kt, :],
                                rhs=w_sb[:, kt, nh * 512 : (nh + 1) * 512],
                                start=(kt == 0),
                                stop=(kt == KT - 1),
                            )
                        col = br * D + nh * 512
                        nc.vector.tensor_add(
                            out_sb[:, col : col + 512], ps[:], bias_bc[:, col : col + 512]
                        )
                nc.sync.dma_start(out[i * P : (i + 1) * P, :], out_sb[:])
            if not xbar:
                tpsum.release()

    nc.compile()
    return nc


def _get_nc(bc=BC):
    if bc not in _NC_CACHE:
        _NC_CACHE[bc] = _build_nc(bc)
    return _NC_CACHE[bc]


def _fuse_weights(w_in, b_in, w_out, b_out):
    """Collapse V-projection + output projection into one [D, D] weight."""
    import ml_dtypes

    wv = np.asarray(w_in, dtype=np.float32)[2 * D : 3 * D]
    bv = np.asarray(b_in, dtype=np.float32)[2 * D : 3 * D]
    w_eff = np.asarray(w_out, dtype=np.float32) @ wv
    b_eff = np.asarray(w_out, dtype=np.float32) @ bv + np.asarray(b_out, dtype=np.float32)
    # Device wants W_eff.T tiled K-major: [p, ko, n] = W_eff.T[ko*P + p, n]
    w_t = np.ascontiguousarray(
        w_eff.T.reshape(KT, P, D).transpose(1, 0, 2)
    ).astype(ml_dtypes.bfloat16)
    return w_t, b_eff


def kernel(
    feat_a,
    feat_b,
    w_in_ab,
    b_in_ab,
    w_out_ab,
    b_out_ab,
    w_in_ba,
    b_in_ba,
    w_out_ba,
    b_out_ba,
):
    global LAST_EXEC_TIME_NS, LAST_RESULTS
    from concourse import bass_utils

    feat_a = np.ascontiguousarray(np.asarray(feat_a, dtype=np.float32))
    feat_b = np.ascontiguousarray(np.asarray(feat_b, dtype=np.float32))

    wab_t, bab = _fuse_weights(w_in_ab, b_in_ab, w_out_ab, b_out_ab)
    wba_t, bba = _fuse_weights(w_in_ba, b_in_ba, w_out_ba, b_out_ba)
    bias = np.concatenate([bab, bba]).reshape(1, 2 * D).astype(np.float32)

    nc = _get_nc()

    in_maps = []
    for c in range(N_CORES):
        sl = slice(c * BC, (c + 1) * BC)
        in_maps.append(
            {
                "xa": feat_a[sl],
                "xb": feat_b[sl],
                "wab": wab_t,
                "wba": wba_t,
                "bias": bias,
            }
        )

    trace = os.environ.get("KERNEL_TRACE", "0") == "1"
    res = bass_utils.run_bass_kernel_spmd(
        nc,
        in_maps,
        core_ids=list(range(N_CORES)),
        trace=trace,
    )
    LAST_EXEC_TIME_NS = res.exec_time_ns
    LAST_RESULTS = res

    out = np.empty((B, 2 * D), dtype=np.float32)
    for c in range(N_CORES):
        out[c * BC : (c + 1) * BC] = res.results[c]["out"]
    return out


# revision 5
# speedup vs baseline: 1.7675x; 1.7675x over previous
"""Bass/Trainium2 kernel for nn_CrossAttentionFusion.

The reference is a pair of seq_len==1 multi-head cross-attentions. With a
single key position, softmax over the key axis is identically 1, so
attention reduces to the V projection:

    attended = (kv @ wv.T + bv) @ w_out.T + b_out
             = kv @ (w_out @ wv).T + (w_out @ bv + b_out)

i.e. one [B, D] x [D, D] GEMM per branch (plus a bias), with the two
effective weights computed on the host from the small projection matrices.

Device kernel (per core, data-parallel over batch), per 128-row batch tile:
  - DMA xa/xb tile in (fp32)
  - fp32 -> bf16 cast on the scalar (ACT) engine
  - transpose to K-major via PE identity matmuls (4 per PSUM bank),
    copied back to SBUF on DVE
  - 8-step PSUM-accumulated bf16 matmuls (N=512, fp32 accum) on PE
  - DVE bias-add PSUM->SBUF
  - DMA the [128, 2048] fp32 output tile out
"""

import os

import numpy as np

B, D = 65536, 1024
N_CORES = 8
BC = B // N_CORES  # 8192 rows per core
P = 128
KT = D // P  # 8 k-tiles

# Updated on every run when tracing is enabled via KERNEL_TRACE=1
LAST_EXEC_TIME_NS = None
LAST_RESULTS = None

_NC_CACHE = {}


def _build_nc(bc=BC):
    import concourse.bacc as bacc
    import concourse.mybir as mybir
    import concourse.tile as tile
    from concourse.masks import make_identity

    f32 = mybir.dt.float32
    bf16 = mybir.dt.bfloat16
    n_tiles = bc // P

    nc = bacc.Bacc(
        "TRN2",
        target_bir_lowering=False,
        debug=False,
        enable_asserts=False,
        num_devices=N_CORES,
    )

    xa = nc.dram_tensor("xa", [bc, D], f32, kind="ExternalInput").ap()
    xb = nc.dram_tensor("xb", [bc, D], f32, kind="ExternalInput").ap()
    # wab/wba hold W_eff.T tiled K-major: w[p, ko, n] = W_eff.T[ko*128 + p, n]
    wab = nc.dram_tensor("wab", [P, KT, D], bf16, kind="ExternalInput").ap()
    wba = nc.dram_tensor("wba", [P, KT, D], bf16, kind="ExternalInput").ap()
    bias = nc.dram_tensor("bias", [1, 2 * D], f32, kind="ExternalInput").ap()
    out = nc.dram_tensor("out", [bc, 2 * D], f32, kind="ExternalOutput").ap()

    with tile.TileContext(nc) as tc:
        with (
            tc.tile_pool(name="const", bufs=1) as const_pool,
            tc.tile_pool(name="xin", bufs=4) as xin_pool,
            tc.tile_pool(name="xbf", bufs=3) as xbf_pool,
            tc.tile_pool(name="xt", bufs=2) as xt_pool,
            tc.tile_pool(name="osb", bufs=3) as out_pool,
            tc.tile_pool(name="tpsum", bufs=3, space="PSUM") as tpsum,
            tc.tile_pool(name="opsum", bufs=1, space="PSUM") as opsum,
        ):
            identity = const_pool.tile([P, P], bf16)
            make_identity(nc, identity)
            bias_bc = const_pool.tile([P, 2 * D], f32)
            nc.sync.dma_start(bias_bc[:], bias.to_broadcast((P, 2 * D)))
            wab_sb = const_pool.tile([P, KT, D], bf16)
            nc.sync.dma_start(wab_sb[:], wab)
            wba_sb = const_pool.tile([P, KT, D], bf16)
            nc.sync.dma_start(wba_sb[:], wba)

            for i in range(n_tiles):
                xa_t = xin_pool.tile([P, D], f32, tag="xa", name="xa_t")
                nc.sync.dma_start(xa_t[:], xa[i * P : (i + 1) * P, :])
                xb_t = xin_pool.tile([P, D], f32, tag="xb", name="xb_t")
                nc.sync.dma_start(xb_t[:], xb[i * P : (i + 1) * P, :])
                out_sb = out_pool.tile([P, 2 * D], f32, tag="out", name="out_sb")

                # branch 0 (ab) consumes xb; branch 1 (ba) consumes xa
                x_bfs, xTs = [], []
                for br, x_t in enumerate((xb_t, xa_t)):
                    x_bf = xbf_pool.tile([P, D], bf16, tag=f"xbf{br}", name="x_bf")
                    nc.scalar.copy(x_bf[:], x_t[:])
                    x_bfs.append(x_bf)
                    xTs.append(
                        xt_pool.tile([P, KT, P], bf16, tag=f"xT{br}", name="xT")
                    )
                # Transpose both branches first: the DVE copy-backs for branch
                # 0 complete while PE transposes branch 1, so the matmul
                # groups below never wait on DVE.
                for br in range(2):
                    for half in range(KT // 4):
                        tp = tpsum.tile([P, 4, P], bf16, tag="tp", name="tp")
                        for q in range(4):
                            kt = half * 4 + q
                            nc.tensor.transpose(
                                tp[:, q, :],
                                x_bfs[br][:, kt * P : (kt + 1) * P],
                                identity[:],
                            )
                        nc.vector.tensor_copy(
                            xTs[br][:, half * 4 : (half + 1) * 4, :], tp[:]
                        )
                for br, w_sb in enumerate((wab_sb, wba_sb)):
                    for nh in range(2):
                        ps = opsum.tile([P, 512], f32, tag=f"ps{br}{nh}", name="ps")
                        for kt in range(KT):
                            nc.tensor.matmul(
                                ps[:],
                                lhsT=xTs[br][:, kt, :],
                                rhs=w_sb[:, kt, nh * 512 : (nh + 1) * 512],
                                start=(kt == 0),
                                stop=(kt == KT - 1),
                            )
                        col = br * D + nh * 512
                        nc.vector.tensor_add(
                            out_sb[:, col : col + 512], ps[:], bias_bc[:, col : col + 512]
                        )
                nc.sync.dma_start(out[i * P : (i + 1) * P, :], out_sb[:])

    nc.compile()
    return nc


def _get_nc(bc=BC):
    if bc not in _NC_CACHE:
        _NC_CACHE[bc] = _build_nc(bc)
    return _NC_CACHE[bc]


def _fuse_weights(w_in, b_in, w_out, b_out):
    """Collapse V-projection + output projection into one [D, D] weight."""
    import ml_dtypes

    wv = np.asarray(w_in, dtype=np.float32)[2 * D : 3 * D]
    bv = np.asarray(b_in, dtype=np.float32)[2 * D : 3 * D]
    w_eff = np.asarray(w_out, dtype=np.float32) @ wv
    b_eff = np.asarray(w_out, dtype=np.float32) @ bv + np.asarray(b_out, dtype=np.float32)
    # Device wants W_eff.T tiled K-major: [p, ko, n] = W_eff.T[ko*P + p, n]
    w_t = np.ascontiguousarray(
        w_eff.T.reshape(KT, P, D).transpose(1, 0, 2)
    ).astype(ml_dtypes.bfloat16)
    return w_t, b_eff


def kernel(
    feat_a,
    feat_b,
    w_in_ab,
    b_in_ab,
    w_out_ab,
    b_out_ab,
    w_in_ba,
    b_in_ba,
    w_out_ba,
    b_out_ba,
):
    global LAST_EXEC_TIME_NS, LAST_RESULTS
    from concourse import bass_utils

    feat_a = np.ascontiguousarray(np.asarray(feat_a, dtype=np.float32))
    feat_b = np.ascontiguousarray(np.asarray(feat_b, dtype=np.float32))

    wab_t, bab = _fuse_weights(w_in_ab, b_in_ab, w_out_ab, b_out_ab)
    wba_t, bba = _fuse_weights(w_in_ba, b_in_ba, w_out_ba, b_out_ba)
    bias = np.concatenate([bab, bba]).reshape(1, 2 * D).astype(np.float32)

    nc = _get_nc()

    in_maps = []
    for c in range(N_CORES):
        sl = slice(c * BC, (c + 1) * BC)
        in_maps.append(
            {
                "xa": feat_a[sl],
                "xb": feat_b[sl],
                "wab": wab_t,
                "wba": wba_t,
                "bias": bias,
            }
        )

    trace = os.environ.get("KERNEL_TRACE", "0") == "1"
    res = bass_utils.run_bass_kernel_spmd(
        nc,
        in_maps,
        core_ids=list(range(N_CORES)),
        trace=trace,
    )
    LAST_EXEC_TIME_NS = res.exec_time_ns
    LAST_RESULTS = res

    out = np.empty((B, 2 * D), dtype=np.float32)
    for c in range(N_CORES):
        out[c * BC : (c + 1) * BC] = res.results[c]["out"]
    return out


# revision 7
# speedup vs baseline: 1.8276x; 1.0340x over previous
"""Bass/Trainium2 kernel for nn_CrossAttentionFusion.

The reference is a pair of seq_len==1 multi-head cross-attentions. With a
single key position, softmax over the key axis is identically 1, so
attention reduces to the V projection:

    attended = (kv @ wv.T + bv) @ w_out.T + b_out
             = kv @ (w_out @ wv).T + (w_out @ bv + b_out)

i.e. one [B, D] x [D, D] GEMM per branch (plus a bias), with the two
effective weights computed on the host from the small projection matrices.

Device kernel (per core, data-parallel over batch), per 128-row batch tile:
  - DMA xa/xb tile in (fp32)
  - fp32 -> bf16 cast on the scalar (ACT) engine
  - transpose to K-major via PE identity matmuls (4 per PSUM bank),
    copied back to SBUF on DVE
  - 8-step PSUM-accumulated bf16 matmuls (N=512, fp32 accum) on PE
  - DVE bias-add PSUM->SBUF
  - DMA the [128, 2048] fp32 output tile out
"""

import os

import numpy as np

B, D = 65536, 1024
N_CORES = 8
BC = B // N_CORES  # 8192 rows per core
P = 128
KT = D // P  # 8 k-tiles

# Updated on every run when tracing is enabled via KERNEL_TRACE=1
LAST_EXEC_TIME_NS = None
LAST_RESULTS = None

_NC_CACHE = {}


def _build_nc(bc=BC):
    import concourse.bacc as bacc
    import concourse.mybir as mybir
    import concourse.tile as tile
    from concourse.masks import make_identity

    f32 = mybir.dt.float32
    bf16 = mybir.dt.bfloat16
    n_tiles = bc // P

    nc = bacc.Bacc(
        "TRN2",
        target_bir_lowering=False,
        debug=False,
        enable_asserts=False,
        num_devices=N_CORES,
    )

    xa = nc.dram_tensor("xa", [bc, D], f32, kind="ExternalInput").ap()
    xb = nc.dram_tensor("xb", [bc, D], f32, kind="ExternalInput").ap()
    # wab/wba hold W_eff.T tiled K-major: w[p, ko, n] = W_eff.T[ko*128 + p, n]
    wab = nc.dram_tensor("wab", [P, KT, D], bf16, kind="ExternalInput").ap()
    wba = nc.dram_tensor("wba", [P, KT, D], bf16, kind="ExternalInput").ap()
    bias = nc.dram_tensor("bias", [1, 2 * D], f32, kind="ExternalInput").ap()
    out = nc.dram_tensor("out", [bc, 2 * D], f32, kind="ExternalOutput").ap()

    with tile.TileContext(nc) as tc:
        with (
            tc.tile_pool(name="const", bufs=1) as const_pool,
            tc.tile_pool(name="xin", bufs=4) as xin_pool,
            tc.tile_pool(name="xbf", bufs=3) as xbf_pool,
            tc.tile_pool(name="xt", bufs=2) as xt_pool,
            tc.tile_pool(name="osb", bufs=3) as out_pool,
            tc.tile_pool(name="tpsum", bufs=4, space="PSUM") as tpsum,
            tc.tile_pool(name="opsum", bufs=1, space="PSUM") as opsum,
        ):
            identity = const_pool.tile([P, P], bf16)
            make_identity(nc, identity)

            def issue_in(i):
                xa_t = xin_pool.tile([P, D], f32, tag="xa", name="xa_t")
                nc.sync.dma_start(xa_t[:], xa[i * P : (i + 1) * P, :])
                xb_t = xin_pool.tile([P, D], f32, tag="xb", name="xb_t")
                nc.sync.dma_start(xb_t[:], xb[i * P : (i + 1) * P, :])
                return xa_t, xb_t

            # Prefetch the first two tiles' inputs before the (large) weight
            # and bias preloads so PE can start transposing immediately.
            tiles_in = {0: issue_in(0), 1: issue_in(1)}

            # Weight column-halves needed by the first matmul groups come
            # first; the bias (only needed by the first bias-add) comes last.
            wab_sb = const_pool.tile([P, KT, D], bf16)
            wba_sb = const_pool.tile([P, KT, D], bf16)
            for nh in range(2):
                nc.sync.dma_start(
                    wab_sb[:, :, nh * 512 : (nh + 1) * 512],
                    wab[:, :, nh * 512 : (nh + 1) * 512],
                )
                nc.sync.dma_start(
                    wba_sb[:, :, nh * 512 : (nh + 1) * 512],
                    wba[:, :, nh * 512 : (nh + 1) * 512],
                )
            bias_bc = const_pool.tile([P, 2 * D], f32)
            nc.sync.dma_start(bias_bc[:], bias.to_broadcast((P, 2 * D)))

            for i in range(n_tiles):
                xa_t, xb_t = tiles_in.pop(i)
                out_sb = out_pool.tile([P, 2 * D], f32, tag="out", name="out_sb")

                # branch 0 (ab) consumes xb; branch 1 (ba) consumes xa
                x_bfs, xTs = [], []
                for br, x_t in enumerate((xb_t, xa_t)):
                    x_bf = xbf_pool.tile([P, D], bf16, tag=f"xbf{br}", name="x_bf")
                    nc.scalar.copy(x_bf[:], x_t[:])
                    x_bfs.append(x_bf)
                    xTs.append(
                        xt_pool.tile([P, KT, P], bf16, tag=f"xT{br}", name="xT")
                    )
                # Transpose both branches first: the DVE copy-backs for branch
                # 0 complete while PE transposes branch 1, so the matmul
                # groups below never wait on DVE.
                for br in range(2):
                    for half in range(KT // 4):
                        tp = tpsum.tile([P, 4, P], bf16, tag="tp", name="tp")
                        for q in range(4):
                            kt = half * 4 + q
                            nc.tensor.transpose(
                                tp[:, q, :],
                                x_bfs[br][:, kt * P : (kt + 1) * P],
                                identity[:],
                            )
                        nc.vector.tensor_copy(
                            xTs[br][:, half * 4 : (half + 1) * 4, :], tp[:]
                        )
                for br, w_sb in enumerate((wab_sb, wba_sb)):
                    for nh in range(2):
                        ps = opsum.tile([P, 512], f32, tag=f"ps{br}{nh}", name="ps")
                        for kt in range(KT):
                            nc.tensor.matmul(
                                ps[:],
                                lhsT=xTs[br][:, kt, :],
                                rhs=w_sb[:, kt, nh * 512 : (nh + 1) * 512],
                                start=(kt == 0),
                                stop=(kt == KT - 1),
                            )
                        col = br * D + nh * 512
                        nc.vector.tensor_add(
                            out_sb[:, col : col + 512], ps[:], bias_bc[:, col : col + 512]
                        )
                # Issue the next tile's input DMAs before this tile's output
                # DMA so they aren't queued behind it.
                if i + 2 < n_tiles:
                    tiles_in[i + 2] = issue_in(i + 2)
                nc.sync.dma_start(out[i * P : (i + 1) * P, :], out_sb[:])

    nc.compile()
    return nc


def _get_nc(bc=BC):
    if bc not in _NC_CACHE:
        _NC_CACHE[bc] = _build_nc(bc)
    return _NC_CACHE[bc]


def _fuse_weights(w_in, b_in, w_out, b_out):
    """Collapse V-projection + output projection into one [D, D] weight."""
    import ml_dtypes

    wv = np.asarray(w_in, dtype=np.float32)[2 * D : 3 * D]
    bv = np.asarray(b_in, dtype=np.float32)[2 * D : 3 * D]
    w_eff = np.asarray(w_out, dtype=np.float32) @ wv
    b_eff = np.asarray(w_out, dtype=np.float32) @ bv + np.asarray(b_out, dtype=np.float32)
    # Device wants W_eff.T tiled K-major: [p, ko, n] = W_eff.T[ko*P + p, n]
    w_t = np.ascontiguousarray(
        w_eff.T.reshape(KT, P, D).transpose(1, 0, 2)
    ).astype(ml_dtypes.bfloat16)
    return w_t, b_eff


def kernel(
    feat_a,
    feat_b,
    w_in_ab,
    b_in_ab,
    w_out_ab,
    b_out_ab,
    w_in_ba,
    b_in_ba,
    w_out_ba,
    b_out_ba,
):
    global LAST_EXEC_TIME_NS, LAST_RESULTS
    from concourse import bass_utils

    feat_a = np.ascontiguousarray(np.asarray(feat_a, dtype=np.float32))
    feat_b = np.ascontiguousarray(np.asarray(feat_b, dtype=np.float32))

    wab_t, bab = _fuse_weights(w_in_ab, b_in_ab, w_out_ab, b_out_ab)
    wba_t, bba = _fuse_weights(w_in_ba, b_in_ba, w_out_ba, b_out_ba)
    bias = np.concatenate([bab, bba]).reshape(1, 2 * D).astype(np.float32)

    nc = _get_nc()

    in_maps = []
    for c in range(N_CORES):
        sl = slice(c * BC, (c + 1) * BC)
        in_maps.append(
            {
                "xa": feat_a[sl],
                "xb": feat_b[sl],
                "wab": wab_t,
                "wba": wba_t,
                "bias": bias,
            }
        )

    trace = os.environ.get("KERNEL_TRACE", "0") == "1"
    res = bass_utils.run_bass_kernel_spmd(
        nc,
        in_maps,
        core_ids=list(range(N_CORES)),
        trace=trace,
    )
    LAST_EXEC_TIME_NS = res.exec_time_ns
    LAST_RESULTS = res

    out = np.empty((B, 2 * D), dtype=np.float32)
    for c in range(N_CORES):
        out[c * BC : (c + 1) * BC] = res.results[c]["out"]
    return out
